# revision 12
# baseline (speedup 1.0000x reference)
"""AVID-CMA retrieval scoring kernel for 8 Trainium2 NeuronCores.

Strategy (batch-sharded, banks replicated per core):
  - 512 batch items -> 64 per core; both 500000x128 memory banks are
    replicated into every core's HBM (host-side, untimed).
  - Per (batch, bank): gather the 1024 negative rows with 8 indirect
    DMAs of 128 rows each (SWDGE, one int32 index per partition).
  - Rows land [row, dim]; PE transposes 128x128 tiles into PSUM, DVE/ACT
    copy them back to SBUF as [dim, row] tiles.
  - Dots = matmul(lhsT=[d,2]=(v_b, a_b) scaled by 1/(||.||*TEMP),
    rhs=[d,512] float32r) -> PSUM [2,512]; four consecutive batch items
    share a PSUM tile at partition strips 0/32/64/96 so one copy + two
    strided DMAs drain 4 items at once.
  - pos/y rows (33 per item per bank) are gathered batched across the
    whole core and scored against all 128 targets at once (wide-M
    matmul); the host picks out the needed entries.
"""

import numpy as np

N_MEM = 500000
DIM = 128
BS = 512
POS_K = 32
NUM_NEG = 1024
TEMP = 0.07
NCORES = 8
NB = BS // NCORES            # 64 batch items per core
PY_PER_B = 1 + POS_K         # y + pos rows per item
PY_ROWS = NB * PY_PER_B      # 2112
PY_TILES = (PY_ROWS + 127) // 128   # 17
PY_PAD = PY_TILES * 128      # 2176
NEG_TILES = NUM_NEG // 128   # 8

_cache = {}


def _split_multi_waits(nc, mybir):
    """This walrus build rejects instructions with >1 sync-wait. Hoist all
    but one wait of each multi-wait instruction into standalone
    EventSemaphore instructions right before it on the same engine."""
    ctr = 0
    for f in nc.m.functions:
        for bb in f.blocks:
            out = []
            changed = False
            for inst in bb.instructions:
                si = inst.sync_info
                if si is not None and len(si.on_wait) > 1:
                    waits = list(si.on_wait)
                    for w in waits[:-1]:
                        ctr += 1
                        es = mybir.InstEventSemaphore(name=f"I-waitsplit-{ctr}")
                        es.engine = inst.engine
                        es.sync_info = mybir.SyncInfo(on_wait=[w], on_update=[])
                        out.append(es)
                    inst.sync_info = mybir.SyncInfo(
                        on_wait=[waits[-1]], on_update=list(si.on_update)
                    )
                    changed = True
                out.append(inst)
            if changed:
                bb.instructions = out


def _build_program():
    import concourse.bass as bass
    import concourse.tile as tile
    from concourse import mybir
    from concourse.bass import IndirectOffsetOnAxis
    from concourse.masks import make_identity

    F32 = mybir.dt.float32
    F32R = mybir.dt.float32r
    I32 = mybir.dt.int32

    nc = bass.Bass("TRN2", target_bir_lowering=False, debug=False, num_devices=1)

    v1_t = nc.dram_tensor("v1", [N_MEM, DIM], F32, kind="ExternalInput")
    v2_t = nc.dram_tensor("v2", [N_MEM, DIM], F32, kind="ExternalInput")
    emb_t = nc.dram_tensor("emb", [2 * NB, DIM], F32, kind="ExternalInput")
    negidx_t = nc.dram_tensor("negidx", [NB, 128, NEG_TILES], I32, kind="ExternalInput")
    pyidx_t = nc.dram_tensor("pyidx", [128, PY_TILES], I32, kind="ExternalInput")

    negd1_t = nc.dram_tensor("negd1", [NB, 2, NUM_NEG], F32, kind="ExternalOutput")
    negd2_t = nc.dram_tensor("negd2", [NB, 2, NUM_NEG], F32, kind="ExternalOutput")
    pyd1_t = nc.dram_tensor("pyd1", [128, PY_PAD], F32, kind="ExternalOutput")
    pyd2_t = nc.dram_tensor("pyd2", [128, PY_PAD], F32, kind="ExternalOutput")

    banks = [v1_t, v2_t]
    negd = [negd1_t, negd2_t]
    pyd = [pyd1_t, pyd2_t]

    with tile.TileContext(nc) as tc:
        with tc.tile_pool(name="const", bufs=1) as cpool:
            ident = cpool.tile([128, 128], F32)
            make_identity(nc, ident[:])
            tgt = cpool.tile([128, 128], F32R)  # [d, tgt]: tgt b = video, 64+b audio
            tgt_f = cpool.tile([128, 128], F32)
            # per-strip target matrices for the neg phase: tsl[i] is all
            # zeros except columns 32i, 32i+1 (item i of the current group)
            tsl = [
                cpool.tile([128, 128], F32R, name=f"tsl{i}") for i in range(4)
            ]
            zsrc = cpool.tile([128, 128], F32)
            nc.gpsimd.memset(zsrc[:], 0)
            for i in range(4):
                nc.vector.tensor_copy(tsl[i][:], zsrc[:])

            # ---- normalize embeddings, fold in 1/TEMP, transpose to [d, tgt]
            with tc.tile_pool(name="psum_n", bufs=1, space="PSUM") as pnorm:
                emb = cpool.tile([128, DIM], F32)
                nc.sync.dma_start(emb[:], emb_t.ap())
                ss = cpool.tile([128, 1], F32)
                sq_scratch = cpool.tile([128, DIM], F32)
                nc.scalar.activation(
                    sq_scratch[:], emb[:], mybir.ActivationFunctionType.Square,
                    accum_out=ss[:],
                )
                nrm = cpool.tile([128, 1], F32)
                nc.scalar.sqrt(nrm[:], ss[:])
                nc.vector.tensor_scalar_max(nrm[:], nrm[:], 1e-12)
                inv = cpool.tile([128, 1], F32)
                nc.vector.reciprocal(inv[:], nrm[:])
                nc.vector.tensor_scalar_mul(inv[:], inv[:], 1.0 / TEMP)
                emb_n = cpool.tile([128, DIM], F32)
                nc.vector.tensor_scalar_mul(emb_n[:], emb[:], inv[:])
                tps = pnorm.tile([128, 128], F32)
                nc.tensor.transpose(tps[:], emb_n[:], identity=ident[:])
                nc.vector.tensor_copy(tgt[:], tps[:])
                nc.scalar.copy(tgt_f[:], tps[:])

            # ---- pos/y phase: 17 tiles x 2 banks, wide-M dots
            with (
                tc.tile_pool(name="pyidx", bufs=1) as pyip,
                tc.tile_pool(name="pyg", bufs=2) as pygp,
                tc.tile_pool(name="pygt", bufs=2) as pygtp,
                tc.tile_pool(name="pyout", bufs=2) as pyop,
                tc.tile_pool(name="psum_pt", bufs=4, space="PSUM") as pypt,
                tc.tile_pool(name="psum_pd", bufs=2, space="PSUM") as pypd,
            ):
                pyidx = pyip.tile([128, PY_TILES], I32)
                nc.sync.dma_start(pyidx[:], pyidx_t.ap())
                for bank in range(2):
                    gpy = pygp.tile([128, PY_TILES, 128], F32)
                    for t in range(PY_TILES):
                        nc.gpsimd.indirect_dma_start(
                            out=gpy[:, t], out_offset=None,
                            in_=banks[bank].ap(),
                            in_offset=IndirectOffsetOnAxis(
                                ap=pyidx[:, t:t + 1], axis=0),
                        )
                    gtpy = pygtp.tile([128, PY_PAD], F32R)
                    pydots = pyop.tile([128, PY_PAD], F32)
                    n_chunks = (PY_TILES + 3) // 4  # 5 chunks: 4x512 + 1x128
                    for c in range(n_chunks):
                        t0 = 4 * c
                        ntile = min(4, PY_TILES - t0)
                        gt_ps = pypt.tile([128, 512], F32)
                        for jj in range(ntile):
                            nc.tensor.transpose(
                                gt_ps[:, 128 * jj:128 * (jj + 1)],
                                gpy[:, t0 + jj], identity=ident[:],
                            )
                        dst = gtpy[:, 128 * t0:128 * (t0 + ntile)]
                        if c % 2 == 0:
                            nc.vector.tensor_copy(dst, gt_ps[:, :128 * ntile])
                        else:
                            nc.scalar.copy(dst, gt_ps[:, :128 * ntile])
                        pydots_ps = pypd.tile([128, 512], F32)
                        nc.tensor.matmul(
                            out=pydots_ps[:, :128 * ntile],
                            lhsT=tgt[:],
                            rhs=gtpy[:, 128 * t0:128 * (t0 + ntile)],
                            start=True, stop=True,
                        )
                        dd = pydots[:, 128 * t0:128 * (t0 + ntile)]
                        if c % 2 == 0:
                            nc.scalar.copy(dd, pydots_ps[:, :128 * ntile])
                        else:
                            nc.vector.tensor_copy(dd, pydots_ps[:, :128 * ntile])
                    nc.sync.dma_start(pyd[bank].ap(), pydots[:])

            # ---- negatives: 64 items x 2 banks x 8 tiles
            with (
                tc.tile_pool(name="nidx", bufs=6) as nip,
                tc.tile_pool(name="g", bufs=10) as gp,
                tc.tile_pool(name="gt", bufs=6) as gtp,
                tc.tile_pool(name="dout", bufs=3) as dop,
                tc.tile_pool(name="psum_t", bufs=4, space="PSUM") as ptp,
                tc.tile_pool(name="psum_d", bufs=2, space="PSUM") as pdp,
            ):
                for grp in range(NB // 4):
                    dps = [
                        pdp.tile([128, NUM_NEG], F32, name=f"dps_b{bk}", tag="dps")
                        for bk in range(2)
                    ]
                    for i in range(4):
                        b = 4 * grp + i
                        # refresh this strip's target columns (rest stays 0)
                        nc.vector.tensor_copy(
                            tsl[i][:, 32 * i:32 * i + 2], tgt_f[:, b::NB]
                        )
                        it = nip.tile([128, NEG_TILES], I32)
                        nc.sync.dma_start(it[:], negidx_t.ap()[b])
                        for bank in range(2):
                            g = gp.tile([128, NEG_TILES, 128], F32)
                            for j in range(NEG_TILES):
                                nc.gpsimd.indirect_dma_start(
                                    out=g[:, j], out_offset=None,
                                    in_=banks[bank].ap(),
                                    in_offset=IndirectOffsetOnAxis(
                                        ap=it[:, j:j + 1], axis=0),
                                )
                            for c in range(2):
                                gt_ps = ptp.tile([128, 512], F32)
                                for jj in range(4):
                                    nc.tensor.transpose(
                                        gt_ps[:, 128 * jj:128 * (jj + 1)],
                                        g[:, 4 * c + jj], identity=ident[:],
                                    )
                                gt_sb = gtp.tile([128, 512], F32R)
                                if (bank + c) % 2 == 0:
                                    nc.vector.tensor_copy(gt_sb[:], gt_ps[:])
                                else:
                                    nc.scalar.copy(gt_sb[:], gt_ps[:])
                                nc.tensor.matmul(
                                    out=dps[bank][:, 512 * c:512 * (c + 1)],
                                    lhsT=tsl[i][:],
                                    rhs=gt_sb[:],
                                    start=(i == 0), stop=(i == 3),
                                )
                    for bank in range(2):
                        dsb = dop.tile([128, NUM_NEG], F32)
                        if bank == 0:
                            nc.vector.tensor_copy(dsb[:], dps[bank][:])
                        else:
                            nc.scalar.copy(dsb[:], dps[bank][:])
                        # partitions {32i+t} hold item 4*grp+i, target-row t
                        for t in range(2):
                            nc.sync.dma_start(
                                negd[bank].ap()[4 * grp:4 * grp + 4, t],
                                dsb[t::32, :],
                            )

    _split_multi_waits(nc, mybir)
    return nc


def _host_prep(video_emb, audio_emb, view1_mem, view2_mem, y, pos_idx, neg_idx):
    """Build per-core input maps."""
    video_emb = np.ascontiguousarray(video_emb, dtype=np.float32)
    audio_emb = np.ascontiguousarray(audio_emb, dtype=np.float32)
    view1_mem = np.ascontiguousarray(view1_mem, dtype=np.float32)
    view2_mem = np.ascontiguousarray(view2_mem, dtype=np.float32)
    y = np.asarray(y).astype(np.int32)
    pos_idx = np.asarray(pos_idx).astype(np.int32)
    neg_idx = np.asarray(neg_idx).astype(np.int32)

    in_maps = []
    for c in range(NCORES):
        sl = slice(c * NB, (c + 1) * NB)
        emb = np.concatenate([video_emb[sl], audio_emb[sl]], axis=0)
        # negidx[b, p, j] = neg_idx[b, 128j + p]
        negidx = np.ascontiguousarray(
            neg_idx[sl].reshape(NB, NEG_TILES, 128).transpose(0, 2, 1)
        )
        # pos/y list per item: [y_b, pos_b(32)]; pyidx[p, t] = L[128t + p]
        L = np.concatenate([y[sl, None], pos_idx[sl]], axis=1).reshape(-1)
        L = np.concatenate([L, np.zeros(PY_PAD - PY_ROWS, np.int32)])
        pyidx = np.ascontiguousarray(L.reshape(PY_TILES, 128).T).astype(np.int32)
        in_maps.append({
            "v1": view1_mem, "v2": view2_mem, "emb": emb,
            "negidx": negidx, "pyidx": pyidx,
        })
    return in_maps


def kernel(video_emb, audio_emb, view1_mem, view2_mem, y, pos_idx, neg_idx):
    from concourse.bass_utils import run_bass_kernel_spmd

    if "nc" not in _cache:
        _cache["nc"] = _build_program()
    nc = _cache["nc"]

    in_maps = _host_prep(video_emb, audio_emb, view1_mem, view2_mem,
                         y, pos_idx, neg_idx)
    res = run_bass_kernel_spmd(nc, in_maps, core_ids=list(range(NCORES)))

    inst_v2a_pos = np.empty((BS, 1), np.float32)
    inst_v2a_neg = np.empty((BS, NUM_NEG), np.float32)
    inst_a2v_pos = np.empty((BS, 1), np.float32)
    inst_a2v_neg = np.empty((BS, NUM_NEG), np.float32)
    pos_v2v_pos = np.empty((BS, POS_K), np.float32)
    pos_v2v_neg = np.empty((BS, NUM_NEG), np.float32)
    pos_a2a_pos = np.empty((BS, POS_K), np.float32)
    pos_a2a_neg = np.empty((BS, NUM_NEG), np.float32)

    bl = np.arange(NB)
    ycol = bl * PY_PER_B          # column of item b's y row
    poscol = bl[:, None] * PY_PER_B + 1 + np.arange(POS_K)[None, :]
    for c in range(NCORES):
        r = res.results[c]
        sl = slice(c * NB, (c + 1) * NB)
        pos_v2v_neg[sl] = r["negd1"][:, 0]
        inst_a2v_neg[sl] = r["negd1"][:, 1]
        inst_v2a_neg[sl] = r["negd2"][:, 0]
        pos_a2a_neg[sl] = r["negd2"][:, 1]
        # pyd rows: partition b = video target, 64+b = audio target
        pyd1, pyd2 = r["pyd1"], r["pyd2"]
        inst_a2v_pos[sl, 0] = pyd1[NB + bl, ycol]      # dot(a, view1[y])
        inst_v2a_pos[sl, 0] = pyd2[bl, ycol]           # dot(v, view2[y])
        pos_v2v_pos[sl] = pyd1[bl[:, None], poscol]    # dot(v, view1[pos])
        pos_a2a_pos[sl] = pyd2[NB + bl[:, None], poscol]

    return (inst_v2a_pos, inst_v2a_neg, inst_a2v_pos, inst_a2v_neg,
            pos_v2v_pos, pos_v2v_neg, pos_a2a_pos, pos_a2a_neg)
